# revision 17
# baseline (speedup 1.0000x reference)
"""Trainium2 Bass kernel for nn_Decoder: greedy GRU decoder with attention.

Strategy (8 NeuronCores, SPMD):
  - Vocab-shard the [2H -> V] output projection: each core holds W^T for its
    4000-entry vocab slice in SBUF (fp32, loaded once) and computes biased
    scores for the full batch each step.
  - GRU gates + attention are computed feature-major (W-stationary matmuls),
    so the hidden state h^T and context ctx^T come out directly in the
    lhsT layout the score matmul needs -- no per-step transposes.
  - The greedy argmax feedback couples vocab shards, so each step does ONE
    AllGather carrying: per-core argmax candidates (top-1 of each of 8
    partition-groups x 32 batch rows) + this core's unnormalized attention
    context columns + softmax denominators.  Every core then reduces the
    64 candidates per row to the global argmax token, gathers that token's
    precomputed gate-row G[tok] = emb[tok] @ Wih_e^T + biases from DRAM via
    dma_gather, and assembles the full ctx^T.
  - log_softmax is computed on the host from the exact fp32 scores
    (mathematically exact normalization; argmax on device is what must be
    bit-faithful, and fp32 matmuls keep score error ~1e-7 relative).

All matmuls are true fp32 (4 cycles/row): TF32 (float32r) or bf16 would
perturb scores by ~1e-3..5e-4 sigma which flips greedy argmaxes and
diverges the whole sequence.
"""
import sys
sys.path.insert(0, '/opt/trn_rl_repo')

import numpy as np
from contextlib import ExitStack

import concourse.bass as bass
import concourse.bacc as bacc
import concourse.tile as tile
from concourse import mybir
from concourse.bass_utils import run_bass_kernel_spmd

dt = mybir.dt
ALU = mybir.AluOpType
ACT = mybir.ActivationFunctionType

NCORES = 8
V, D, H = 32000, 512, 512
B, T = 32, 128
NSTEPS = 48
VS = V // NCORES          # 4000 vocab entries per core
NG = 500                  # score N-slice per col-group (WSTAT=False)
WSTAT = True              # W-stationary score (score^T + PE transposes)
VSP = 4096                # padded per-core vocab for WSTAT
RB = B // NCORES          # 4 batch rows per core for attention
GW = 3 * H                # 1536 gate width
CBIG = 32768.0            # index-packing constant (> V)
AGN = 64 + 2048           # exchange payload floats per core

_compiled = None


def _build(nsteps=NSTEPS, dbg=False, gtab_rows=V):
    nc = bacc.Bacc("TRN2", target_bir_lowering=False, debug=False,
                   num_devices=NCORES)

    # ---------------- DRAM parameters ----------------
    wt = nc.dram_tensor("wt", [2 * H, VSP if WSTAT else VS], dt.float32, kind="ExternalInput")
    bias = nc.dram_tensor("bias", [128, 2, 512 if WSTAT else NG], dt.float32, kind="ExternalInput")
    goff = nc.dram_tensor("goff", [128, 2], dt.float32, kind="ExternalInput")
    wgt = nc.dram_tensor("wgt", [2 * H, GW], dt.float32, kind="ExternalInput")
    gtab = nc.dram_tensor("gtab", [gtab_rows, GW], dt.float32, kind="ExternalInput")
    g0 = nc.dram_tensor("g0", [B, GW], dt.float32, kind="ExternalInput")
    ctx0t = nc.dram_tensor("ctx0t", [H, B], dt.float32, kind="ExternalInput")
    hb4 = nc.dram_tensor("hb4", [128, 4], dt.float32, kind="ExternalInput")
    enc = nc.dram_tensor("enc", [RB, T, H], dt.float32, kind="ExternalInput")
    egt = nc.dram_tensor("egt", [RB, H, T], dt.float32, kind="ExternalInput")
    ident = nc.dram_tensor("ident", [32, 32], dt.float32, kind="ExternalInput")
    ident128 = nc.dram_tensor("ident128", [128, 128], dt.float32, kind="ExternalInput")
    out = nc.dram_tensor("out", [nsteps, B, VS], dt.float32,
                         kind="ExternalOutput")
    if dbg:
        dbg_psg = nc.dram_tensor("dbg_psg", [128, 16, B], dt.float32,
                                 kind="ExternalOutput")
        dbg_h = nc.dram_tensor("dbg_h", [128, 4, B], dt.float32,
                               kind="ExternalOutput")
        dbg_gat = nc.dram_tensor("dbg_gat", [B, GW], dt.float32,
                                 kind="ExternalOutput")
        dbg_rz = nc.dram_tensor("dbg_rz", [128, 8, B], dt.float32,
                                kind="ExternalOutput")
        dbg_n = nc.dram_tensor("dbg_n", [128, 4, B], dt.float32,
                               kind="ExternalOutput")

    with tile.TileContext(nc) as tc, ExitStack() as ctx:
        wp = ctx.enter_context(tc.tile_pool(name="wp", bufs=1))    # persistent
        kp = ctx.enter_context(tc.tile_pool(name="kp", bufs=1))    # working
        sp = ctx.enter_context(tc.tile_pool(name="sp", bufs=2))    # dbl-buffered
        ps = ctx.enter_context(tc.tile_pool(name="ps", bufs=1, space="PSUM"))
        dr = ctx.enter_context(tc.tile_pool(name="dr", bufs=2, space="DRAM"))

        # ---------------- one-time loads ----------------
        WTW = VSP if WSTAT else VS
        wt_sb = wp.tile([128, 7, WTW], dt.float32, tag="wt")
        nc.sync.dma_start(
            wt_sb[:], wt.ap()[0:7 * 128, :].rearrange("(k p) n -> p k n", p=128))
        wg_sb = wp.tile([128, 8, GW], dt.float32, tag="wg")
        nc.sync.dma_start(wg_sb[:], wgt.ap().rearrange("(k p) n -> p k n", p=128))
        bias_sb = wp.tile([128, 2, 512 if WSTAT else NG], dt.float32, tag="bias")
        nc.sync.dma_start(bias_sb[:], bias.ap())
        goff_sb = wp.tile([128, 2], dt.float32, tag="goff")
        nc.sync.dma_start(goff_sb[:], goff.ap())
        enc_sb = wp.tile([128, RB, H], dt.float32, tag="enc")
        nc.sync.dma_start(enc_sb[:], enc.ap().rearrange("r p h -> p r h"))
        egt_sb = wp.tile([128, RB, 4, T], dt.float32, tag="egt")
        nc.sync.dma_start(
            egt_sb[:], egt.ap().rearrange("r (c p) t -> p r c t", p=128))
        hb4_sb = wp.tile([128, 4], dt.float32, tag="hb4")
        nc.sync.dma_start(hb4_sb[:], hb4.ap())
        id_sb = wp.tile([32, 32], dt.float32, tag="id")
        nc.sync.dma_start(id_sb[:], ident.ap())
        if WSTAT:
            id128_sb = wp.tile([128, 128], dt.float32, tag="id128")
            nc.sync.dma_start(id128_sb[:], ident128.ap())
        ones128 = wp.tile([128, 128], dt.float32, tag="ones128")
        nc.vector.memset(ones128[:], 1.0)

        # persistent state
        hT = wp.tile([128, 4, B], dt.float32, tag="hT")
        nc.vector.memset(hT[:], 0.0)
        ctxT = wp.tile([128, 4, B], dt.float32, tag="ctxT")
        nc.sync.dma_start(ctxT[:],
                          ctx0t.ap().rearrange("(c p) b -> p c b", p=128))

        rank = nc.tensor.cc_rank([list(range(NCORES))])
        r4 = rank * RB

        for t in range(nsteps):
            # ====== gate-row input for this step ======
            gat = kp.tile([128, GW], dt.float32, tag="gat")
            if t == 0:
                nc.sync.dma_start(gat[0:B, :], g0.ap())
            else:
                # gidx prepared at the end of the previous step
                nc.gpsimd.dma_gather(
                    out_ap=gat[:].rearrange("p (o w) -> p o w", o=1),
                    in_ap=gtab.ap(),
                    idxs_ap=gidx_sb[:],
                    num_idxs=B,
                    num_idxs_reg=B,
                    elem_size=GW,
                )

            # ====== GRU gates (feature-major, W-stationary) ======
            # psR: r-gates (m 0..3)   = ctx + h + G
            # psZ: z-gates (m 4..7)   = ctx + h + G
            # psI: inn     (m 8..11)  = ctx + G
            # psH: hn      (m 8..11)  = h + bhh_n
            psG = ps.tile([128, 16, B], dt.float32, tag="psG")
            psR = psG[:, 0:4, :]
            psZ = psG[:, 4:8, :]
            psI = psG[:, 8:12, :]
            psH = psG[:, 12:16, :]

            def gate_dst(m):
                return psG[:, m, :]

            # one accumulation group at a time per psum slice (m outer):
            # a start marks the whole 2KB region pending on its partitions,
            # so interleaved groups on shared partitions clobber each other.
            for m in range(12):
                for k in range(4):      # ctx feature chunks
                    nc.tensor.matmul(gate_dst(m),
                                     wg_sb[:, k + 4, 128 * m:128 * (m + 1)],
                                     ctxT[:, k, :],
                                     start=(k == 0), stop=(k == 3 and m >= 8),
                                     skip_group_check=True)
                if m < 8:
                    for k in range(4):  # h feature chunks
                        nc.tensor.matmul(gate_dst(m),
                                         wg_sb[:, k, 128 * m:128 * (m + 1)],
                                         hT[:, k, :],
                                         start=False, stop=(k == 3),
                                         skip_group_check=True)
            for m in range(4):          # hn: h-part only
                for k in range(4):
                    nc.tensor.matmul(psH[:, m, :],
                                     wg_sb[:, k, 128 * m + 1024:128 * (m + 1) + 1024],
                                     hT[:, k, :],
                                     start=(k == 0), stop=(k == 3),
                                     skip_group_check=True)
            # G rows: transpose G[tok] chunks into their own psum, then evac
            psGt = ps.tile([128, 12, B], dt.float32, tag="psGt")
            for m in range(12):
                nc.tensor.transpose(psGt[:, m, :],
                                    gat[0:B, 128 * m:128 * (m + 1)],
                                    id_sb[:])
            gTs = kp.tile([128, 12, B], dt.float32, tag="gTs")
            nc.vector.tensor_copy(gTs[:], psGt[:])

            urz = kp.tile([128, 8, B], dt.float32, tag="urz")
            nc.vector.tensor_add(urz[:], psG[:, 0:8, :], gTs[:, 0:8, :])
            if dbg and t == 0:
                dtmp = kp.tile([128, 16, B], dt.float32, tag="dtmp")
                nc.vector.tensor_copy(dtmp[:], psG[:])
                nc.vector.tensor_add(dtmp[:, 0:8, :], psG[:, 0:8, :],
                                     gTs[:, 0:8, :])
                nc.vector.tensor_add(dtmp[:, 8:12, :], psG[:, 8:12, :],
                                     gTs[:, 8:12, :])
                nc.sync.dma_start(dbg_psg.ap(), dtmp[:])
                nc.sync.dma_start(dbg_gat.ap(), gat[0:B, :])
            # nonlinearities
            rz = kp.tile([128, 8, B], dt.float32, tag="rz")
            nc.scalar.activation(rz[:], urz[:], ACT.Sigmoid)
            rT = rz[:, 0:4, :]
            zT = rz[:, 4:8, :]
            t1 = kp.tile([128, 4, B], dt.float32, tag="t1")
            for m in range(4):
                # t1 = (hn + bhh_n) * r
                nc.vector.scalar_tensor_tensor(
                    t1[:, m, :], psH[:, m, :], hb4_sb[:, m:m + 1],
                    rz[:, m, :],
                    op0=ALU.add, op1=ALU.mult)
            nc.vector.tensor_add(t1[:], t1[:], psI[:])
            nc.vector.tensor_add(t1[:], t1[:], gTs[:, 8:12, :])
            nT = kp.tile([128, 4, B], dt.float32, tag="nT")
            nc.scalar.activation(nT[:], t1[:], ACT.Tanh)
            if dbg and t == 0:
                nc.sync.dma_start(dbg_rz.ap(), rz[:])
                nc.sync.dma_start(dbg_n.ap(), nT[:])
            dT = kp.tile([128, 4, B], dt.float32, tag="dT")
            nc.vector.tensor_sub(dT[:], hT[:], nT[:])
            nc.vector.tensor_mul(dT[:], zT, dT[:])
            nc.vector.tensor_add(hT[:], nT[:], dT[:])   # h' in place
            if dbg and t == 0:
                nc.sync.dma_start(dbg_h.ap(), hT[:])

            # ====== score = [h', ctx] @ W^T + b  (x-stationary, col-tiled) ======
            for s in range(2):
                if WSTAT:
                    # score^T: W-stationary, 16 vocab chunks of 128 per half
                    psT = ps.tile([128, 16, B], dt.float32, tag=f"psT{s}")
                    for q in range(4):
                        wst = sp.tile([128, 512], dt.float32, tag="wstream")
                        nc.sync.dma_start(
                            wst[:],
                            wt.ap()[896:1024,
                                    2048 * s + 512 * q:2048 * s + 512 * (q + 1)])
                        for ml in range(4 * q, 4 * q + 4):
                            mg = 2048 * s + 128 * ml
                            for k in range(8):
                                src = hT if k < 4 else ctxT
                                lhsT = (wst[:, 128 * (ml % 4):128 * (ml % 4 + 1)]
                                        if k == 7 else
                                        wt_sb[:, k, mg:mg + 128])
                                nc.tensor.matmul(
                                    psT[:, ml, :], lhsT, src[:, k % 4, :],
                                    start=(k == 0), stop=(k == 7),
                                    skip_group_check=True)
                    sT = kp.tile([128, 16, B], dt.float32, tag=f"sT{s}")
                    nc.vector.tensor_copy(sT[:], psT[:])
                    stage = sp.tile([128, 512], dt.float32, tag="stage")
                    for g in range(4):
                        ps2 = ps.tile([32, 4, 128], dt.float32, tag="ps2")
                        for j in range(4):
                            nc.tensor.transpose(ps2[:, j, :],
                                                sT[:, 4 * g + j, :],
                                                id128_sb[:])
                        nc.vector.scalar_tensor_tensor(
                            stage[32 * g:32 * (g + 1), :],
                            ps2[:].rearrange("b j n -> b (j n)"), 1.0,
                            bias_sb[32 * g:32 * (g + 1), s, :],
                            op0=ALU.mult, op1=ALU.add)
                    # output: vocab [2048s, 2048s+2048), clipped to 4000
                    if s == 0:
                        nc.sync.dma_start(
                            out.ap()[t][:, 0:2048]
                               .rearrange("b (g j) -> b g j", g=4)
                               .transpose([1, 0, 2]),
                            stage[:])
                    else:
                        nc.sync.dma_start(
                            out.ap()[t][:, 2048:3584]
                               .rearrange("b (g j) -> b g j", g=3)
                               .transpose([1, 0, 2]),
                            stage[0:96, :])
                        nc.sync.dma_start(
                            out.ap()[t][:, 3584:4000],
                            stage[96:128, 0:416])
                else:
                    pst = ps.tile([128, 512], dt.float32, tag=f"scps{s}")
                    for g in range(4):
                        lo = 2000 * s + NG * g
                        wst = sp.tile([128, NG], dt.float32, tag="wstream")
                        nc.sync.dma_start(wst[:], wt.ap()[896:1024, lo:lo + NG])
                        for k in range(8):
                            src = hT if k < 4 else ctxT
                            rhs = wst[:] if k == 7 else wt_sb[:, k, lo:lo + NG]
                            nc.tensor.matmul(
                                pst[32 * g:32 * (g + 1), 0:NG],
                                src[:, k % 4, :],
                                rhs,
                                start=(k == 0), stop=(k == 7),
                                tile_position=(0, 32 * g),
                                skip_group_check=True)
                    stage = sp.tile([128, NG], dt.float32, tag="stage")
                    nc.vector.scalar_tensor_tensor(
                        stage[:], pst[:, 0:NG], 1.0, bias_sb[:, s, :],
                        op0=ALU.mult, op1=ALU.add)
                    nc.sync.dma_start(
                        out.ap()[t][:, 2000 * s:2000 * (s + 1)]
                           .rearrange("b (g j) -> b g j", g=4).transpose([1, 0, 2]),
                        stage[:])
                if t == nsteps - 1:
                    continue
                mx = kp.tile([128, 8], dt.float32, tag=f"mx{s}")
                mi = kp.tile([128, 8], dt.uint32, tag=f"mi{s}")
                nc.vector.max_with_indices(mx[:], mi[:], stage[:])
                if s == 0:
                    payc = kp.tile([128, 4], dt.float32, tag="payc")
                nc.vector.tensor_copy(payc[:, 2 * s:2 * s + 1], mx[:, 0:1])
                mif = kp.tile([128, 1], dt.float32, tag=f"mif{s}")
                nc.vector.tensor_copy(mif[:], mi[:, 0:1])
                nc.vector.tensor_scalar(
                    payc[:, 2 * s + 1:2 * s + 2], mif[:],
                    goff_sb[:, s:s + 1], None, op0=ALU.add)

            if t == nsteps - 1:
                break       # no feedback needed after the last step

            # ====== attention (this core's 4 batch rows) ======
            psL = ps.tile([128, RB], dt.float32, tag="psL")
            for r in range(RB):
                for k in range(4):
                    nc.tensor.matmul(psL[:, r:r + 1],
                                     egt_sb[:, r, k, :],
                                     hT[:, k, bass.ds(r4 + r, 1)],
                                     start=(k == 0), stop=(k == 3),
                                     skip_group_check=True)
            expl = kp.tile([128, RB], dt.float32, tag="expl")
            nc.scalar.activation(expl[:], psL[:], ACT.Exp)
            psA = ps.tile([128, 5, RB], dt.float32, tag="psA")
            psC = psA[:, 0:4, :]
            psDr = psA[:, 4, :]
            for r in range(RB):
                for c in range(4):
                    nc.tensor.matmul(psC[:, c, r:r + 1],
                                     enc_sb[:, r, 128 * c:128 * (c + 1)],
                                     expl[:, r:r + 1],
                                     start=True, stop=True,
                                     skip_group_check=True)
                # denominator replicated on all partitions: ones^T @ expl
                nc.tensor.matmul(psDr[:, r:r + 1],
                                 ones128[:], expl[:, r:r + 1],
                                 start=True, stop=True,
                                 skip_group_check=True)
            recipd = kp.tile([128, RB], dt.float32, tag="recipd")
            nc.vector.reciprocal(recipd[:], psDr[:])
            payn = kp.tile([128, 4, RB], dt.float32, tag="payn")
            for c in range(4):
                nc.vector.tensor_mul(payn[:, c, :], psC[:, c, :], recipd[:])

            # ====== local candidate combine (4 groups x 2 stages -> top-1) ======
            cndb = dr.tile([B, 16], dt.float32, tag="cndb")
            nc.sync.dma_start(
                bass.AP(cndb.tensor, 0, [[4, 4], [16, B], [1, 4]]), payc[:])
            cnd = kp.tile([B, 16], dt.float32, tag="cnd")
            nc.sync.dma_start(cnd[:], cndb[:])
            cndv = cnd[:, 0:16:2]
            cndi = cnd[:, 1:16:2]
            m1 = kp.tile([B, 1], dt.float32, tag="m1")
            nc.vector.tensor_reduce(m1[:], cndv,
                                    axis=mybir.AxisListType.X, op=ALU.max)
            msk1 = kp.tile([B, 8], dt.float32, tag="msk1")
            nc.vector.tensor_scalar(msk1[:], cndv, m1[:], None,
                                    op0=ALU.is_equal)
            ci1 = kp.tile([B, 8], dt.float32, tag="ci1")
            nc.vector.tensor_scalar(ci1[:], cndi, CBIG, None, op0=ALU.subtract)
            nc.vector.tensor_mul(msk1[:], msk1[:], ci1[:])
            r1 = kp.tile([B, 1], dt.float32, tag="r1")
            nc.vector.tensor_reduce(r1[:], msk1[:],
                                    axis=mybir.AxisListType.X, op=ALU.min)
            pay1 = kp.tile([B, 2], dt.float32, tag="pay1")
            nc.vector.tensor_copy(pay1[:, 0:1], m1[:])
            nc.vector.tensor_scalar(pay1[:, 1:2], r1[:], CBIG, None,
                                    op0=ALU.add)

            # ====== exchange ======
            agin = dr.tile([AGN], dt.float32, tag="agin")
            agout = dr.tile([NCORES, AGN], dt.float32, tag="agout")
            nc.sync.dma_start(
                bass.AP(agin.tensor, 0, [[2, B], [1, 2]]), pay1[:])
            nc.sync.dma_start(
                bass.AP(agin.tensor, 64, [[16, 128], [4, 4], [1, RB]]), payn[:])

            nc.gpsimd.collective_compute(
                "AllGather", ALU.bypass,
                replica_groups=[list(range(NCORES))],
                ins=[agin[:].opt()],
                outs=[agout[:].opt()],
            )
            # read back: candidates [b, core, (val, idx)]
            vals = kp.tile([B, 8, 2], dt.float32, tag="vals")
            nc.sync.dma_start(
                vals[:],
                bass.AP(agout.tensor, 0, [[2, B], [AGN, 8], [1, 2]]))
            # read back: ctx columns + denominators
            for c in range(4):
                nc.sync.dma_start(
                    ctxT[:, c, :].rearrange("p (i r) -> p i r", i=8).opt(),
                    bass.AP(agout.tensor, 64 + 4 * c,
                            [[16, 128], [AGN, 8], [1, RB]]))


            # ====== global argmax -> token -> gather indices ======
            valsf = vals[:, :, 0]
            idxff = vals[:, :, 1]
            mrow = kp.tile([B, 1], dt.float32, tag="mrow")
            nc.vector.tensor_reduce(mrow[:], valsf,
                                    axis=mybir.AxisListType.X, op=ALU.max)
            mask = kp.tile([B, 8], dt.float32, tag="mask")
            nc.vector.tensor_scalar(mask[:], valsf, mrow[:], None,
                                    op0=ALU.is_equal)
            c1 = kp.tile([B, 8], dt.float32, tag="c1")
            nc.vector.tensor_scalar(c1[:], idxff, CBIG, None,
                                    op0=ALU.subtract)
            nc.vector.tensor_mul(mask[:], mask[:], c1[:])
            rmin = kp.tile([B, 1], dt.float32, tag="rmin")
            nc.vector.tensor_reduce(rmin[:], mask[:],
                                    axis=mybir.AxisListType.X, op=ALU.min)
            tokf = kp.tile([B, 1], dt.float32, tag="tokf")
            nc.vector.tensor_scalar(tokf[:], rmin[:], CBIG, None, op0=ALU.add)
            tok16 = kp.tile([B, 1], dt.int16, tag="tok16")
            nc.vector.tensor_copy(tok16[:], tokf[:])
            tokd = dr.tile([B], dt.int16, tag="tokd")
            nc.sync.dma_start(tokd[:], tok16[:])
            gidx_sb = kp.tile([128, 2], dt.int16, tag="gidx")
            src_fold = bass.AP(tokd.tensor, 0, [[1, 16], [16, 2]])
            for kk in range(8):
                nc.sync.dma_start(gidx_sb[16 * kk:16 * (kk + 1), :], src_fold)

    nc.compile()
    return nc


def _host_prep(inputs, context, max_len, encoder_outputs, emb, gru_Wih,
               gru_Whh, gru_bih, gru_bhh, lin_W, lin_b, attn_W, attn_b):
    """Build the per-core input maps (all heavy math in float64)."""
    emb64 = emb.astype(np.float64)
    Wih64 = gru_Wih.astype(np.float64)
    gtab = (emb64 @ Wih64[:, :D].T + gru_bih.astype(np.float64)[None, :])
    bhh = gru_bhh.astype(np.float64)
    gtab[:, :2 * H] += bhh[None, :2 * H]
    gtab = np.ascontiguousarray(gtab, dtype=np.float32).astype(np.float32)

    wgt = np.concatenate([gru_Whh, gru_Wih[:, D:]], axis=1)  # [1536, h|ctx]
    # rows of lhsT: k 0..3 -> h features, 4..7 -> ctx features
    wgt_T = np.ascontiguousarray(
        np.concatenate([gru_Whh.T, gru_Wih[:, D:].T], axis=0),
        dtype=np.float32)                                     # [1024, 1536]

    energies = (encoder_outputs.astype(np.float64) @
                attn_W.astype(np.float64).T +
                attn_b.astype(np.float64)[None, None, :]).astype(np.float32)

    tok0 = np.asarray(inputs)[:, 0].astype(np.int64)
    g0 = gtab[tok0]                                           # [B, 1536]
    ctx0 = np.asarray(context)[:, 0, :].astype(np.float32)
    ctx0t = np.ascontiguousarray(ctx0.T)                      # [H, B]
    hb4 = np.ascontiguousarray(
        bhh[2 * H:].astype(np.float32).reshape(4, 128).T)
    enc32 = np.asarray(encoder_outputs, dtype=np.float32)

    in_maps = []
    for c in range(NCORES):
        wsh = lin_W[VS * c:VS * (c + 1), :]                   # [4000, 1024]
        wt = np.ascontiguousarray(wsh.T, dtype=np.float32)    # [1024, 4000]
        bsh = lin_b[VS * c:VS * (c + 1)].astype(np.float32)
        if WSTAT:
            wt = np.concatenate(
                [wt, np.zeros((2 * H, VSP - VS), np.float32)], axis=1)
            bsh = np.concatenate(
                [bsh, np.full((VSP - VS,), -1e30, np.float32)])
            NGW = 512
        else:
            NGW = NG
        bias_exp = np.zeros((128, 2, NGW), np.float32)
        goff = np.zeros((128, 2), np.float32)
        for s in range(2):
            for g in range(4):
                lo = (2048 if WSTAT else 2000) * s + NGW * g
                bias_exp[32 * g:32 * (g + 1), s, :] = bsh[lo:lo + NGW][None, :]
                goff[32 * g:32 * (g + 1), s] = VS * c + lo
        rows = slice(RB * c, RB * (c + 1))
        enc_c = np.ascontiguousarray(enc32[rows])             # [4, 128, 512]
        egt_c = np.ascontiguousarray(
            energies[rows].transpose(0, 2, 1))                # [4, 512, 128]
        in_maps.append(dict(
            wt=wt, bias=bias_exp, goff=goff, wgt=wgt_T, gtab=gtab, g0=g0,
            ctx0t=ctx0t, hb4=hb4, enc=enc_c, egt=egt_c,
            ident=np.eye(32, dtype=np.float32),
            ident128=np.eye(128, dtype=np.float32)))
    return in_maps


def kernel(**inputs) -> np.ndarray:
    global _compiled
    nsteps = int(inputs["max_len"])
    in_maps = _host_prep(**inputs)
    if _compiled is None or _compiled[0] != nsteps:
        _compiled = (nsteps, _build(nsteps))
    nc = _compiled[1]
    try:
        res = run_bass_kernel_spmd(nc, in_maps, core_ids=list(range(NCORES)))
    except Exception:
        # transient axon worker failures: retry once
        import time as _time
        _time.sleep(5)
        res = run_bass_kernel_spmd(nc, in_maps, core_ids=list(range(NCORES)))
    # assemble: per-core out [S, B, VS] -> [S, B, V]
    scores = np.concatenate([res.results[c]["out"] for c in range(NCORES)],
                            axis=2)
    scores = scores.transpose(1, 0, 2).reshape(B * nsteps, V)
    s64 = scores.astype(np.float64)
    m = s64.max(axis=1, keepdims=True)
    lse = m + np.log(np.exp(s64 - m).sum(axis=1, keepdims=True))
    return (s64 - lse).astype(np.float32)
